# revision 3
# baseline (speedup 1.0000x reference)
"""ChebConvolution (K=4) Trainium2 kernel, 8-way sharded.

Math: with P = spmm(2*adj_vals) and right-multiplication by W commuting
with the (linear) sparse propagation, the reference collapses to

    Y = P(table2) - V,   table2 = Z1 @ W^3,  Z1 = P X,
                         V = Z1 @ W^3 + X @ W^2

Per core c (rows [c*S, (c+1)*S)):
  phase 1: SpMM Z1 rows via dma_gather from replicated bf16 X table +
           host-precomputed val-at-dest mask slabs streamed from HBM,
           accumulated in PSUM (feature-major) via mask matmuls,
           then t2 = Z1 @ W3 and V = t2 + X @ W2 per 128-row block.
  AllGather t2 shards -> full bf16 table2, CHUNKED (4 chunks, chunk-major
           [chunk, core, rows, D] layout) so chunks 0..2 overlap with the
           phase-1 gather stream; phase-2 gather indices are host-remapped
           into the chunk-major space (its own tile schedule + masks).
  phase 2: SpMM P(table2) rows (node-major psum), subtract V, write Y shard.

Edges are sorted by dest; each 128-edge tile maps to an anchor dest block;
its [128, 128] mask slabs (up to 3 per tile for dests in [128*anchor,
128*anchor+384)) hold val[e] at [slot, dest_rel], built ON HOST and DMA-
streamed in consumption order. Q7 SWDGE descriptor generation (~8.4ns/idx,
hard num_idxs<=1024/call, single queue) is the serial bottleneck; mask DMA,
PE matmuls, tails and collectives hide under its shadow.

Edges are partitioned by dest core and split by source half (int16 gather
index limit); per-(core,half) streams are padded only at the end to a
cross-core-uniform tile count so one NEFF serves all 8 cores.
"""

import os
import sys

for _p in ("/opt/trn_rl_repo", "/root/.axon_site/_ro/trn_rl_repo"):
    if os.path.isdir(_p) and _p not in sys.path:
        sys.path.insert(0, _p)

import numpy as np
import ml_dtypes

import concourse.bacc as bacc
import concourse.mybir as mybir
import concourse.tile as tile
from concourse.bass_utils import run_bass_kernel_spmd

F32 = mybir.dt.float32
BF16 = mybir.dt.bfloat16
I16 = mybir.dt.int16

D = 128            # feature dim (in == out == 128)
SPLIT = 32768      # int16 gather index limit -> lo/hi table halves
CH_TILES = 8       # gather chunk: 1024 idx (HW dma_gather limit)
MCH = 8            # mask slabs per DMA chunk
NCH_AG = 4         # AllGather chunks
AG_LOOKAHEAD = 4   # blocks past a chunk end before its AG trigger is emitted


def _pack_idxs(flat_idx):
    """int16 gather index layout: [128, n/16], idx j at [16k + j%16, j//16]."""
    n = len(flat_idx)
    assert n % 16 == 0
    arr = flat_idx.astype(np.int16).reshape(n // 16, 16).T  # [16, n/16]
    return np.tile(arr, (8, 1))


def _build_schedule(ncores, NB, core_s, dloc_s, col_s, val_s, order_key):
    """Group (already dest-sorted per core) edges into per-(core,half) tile
    streams for a given mapped column space; build the union block schedule
    and the host mask slab stream.

    core_s/dloc_s/col_s/val_s are the dest-sorted per-edge arrays; col_s is
    in the target (possibly remapped) table space. order_key re-sorts within
    (core, half) keeping dest order stable.
    Returns (sched dict, per-core dict arrays).
    """
    half = (col_s >= SPLIT).astype(np.int64)

    ch_key = core_s * 2 + half
    cnt = np.bincount(ch_key, minlength=ncores * 2).reshape(ncores, 2)
    T_half = [max(int(-(-cnt[:, h].max() // 128)), 1) for h in (0, 1)]
    if (half == 0).all():
        T_half[1] = 0

    # order within (core, half) by (dest, col); global lexsort
    order = np.lexsort((order_key, dloc_s, half, core_s))
    k_s = ch_key[order]
    firsts = np.r_[0, np.flatnonzero(np.diff(k_s)) + 1]
    seg_of = np.cumsum(np.isin(np.arange(len(k_s)), firsts)) - 1
    rank = np.arange(len(k_s)) - firsts[seg_of]

    core_o, half_o = core_s[order], half[order]
    col_o, dloc_o, val_o = col_s[order], dloc_s[order], val_s[order]
    tile_in_half = rank // 128

    anchors, straddles = [], []
    for h in range(2):
        T = max(T_half[h], 1)
        tmin = np.full(T, 1 << 30, np.int64)
        tmax = np.full(T, -1, np.int64)
        m = half_o == h
        if m.any():
            np.minimum.at(tmin, tile_in_half[m], dloc_o[m])
            np.maximum.at(tmax, tile_in_half[m], dloc_o[m])
        anchor = np.where(tmax >= 0, np.minimum(tmin // 128, NB - 1), 0)
        assert (tmax < anchor * 128 + 384).all(), "tile dest span exceeds 384"
        anchors.append(anchor)
        straddles.append((tmax >= (anchor + 1) * 128,
                          tmax >= (anchor + 2) * 128))

    block_mms = [[] for _ in range(NB)]
    for h in range(2):
        if T_half[h] == 0:
            continue
        for t in range(T_half[h]):
            b = int(anchors[h][t])
            block_mms[b].append((h, t, 0))
            if straddles[h][0][t]:
                block_mms[b + 1].append((h, t, 1))
            if straddles[h][1][t]:
                block_mms[b + 2].append((h, t, 2))
    assert all(block_mms[b] for b in range(NB)), "block with no matmuls"
    mm_ord = {}
    for b in range(NB):
        for key in block_mms[b]:
            mm_ord[key] = len(mm_ord)
    NMM = len(mm_ord)

    idx_streams = [np.zeros((ncores, max(T_half[h], 1) * 128), np.int64)
                   for h in range(2)]
    for h in range(2):
        m = half_o == h
        if not m.any():
            continue
        idx_streams[h][core_o[m], rank[m]] = col_o[m] - (SPLIT if h else 0)

    t0c = np.minimum(tile_in_half, max(T_half[0], 1) - 1)
    t1c = np.minimum(tile_in_half, max(T_half[1], 1) - 1)
    anchor_o = np.where(half_o == 0, anchors[0][t0c], anchors[1][t1c])
    rel = dloc_o - 128 * anchor_o
    assert ((rel >= 0) & (rel < 384)).all()

    lut = np.full((2, max(T_half[0], T_half[1], 1), 3), -1, np.int64)
    for (h, t, sl), o in mm_ord.items():
        lut[h, t, sl] = o
    slab_id = lut[half_o, tile_in_half, rel // 128]
    assert (slab_id >= 0).all()

    mask_all = np.zeros((ncores, NMM * 128 * 128), ml_dtypes.bfloat16)
    mpos = slab_id * (128 * 128) + (rank % 128) * 128 + (rel % 128)
    mask_all[core_o, mpos] = val_o.astype(ml_dtypes.bfloat16)

    sched = dict(NB=NB, T_lo=T_half[0], T_hi=T_half[1], NMM=NMM,
                 block_mms=block_mms)
    per_core = []
    for c in range(ncores):
        mk = np.ascontiguousarray(
            mask_all[c].reshape(NMM, 128, 128).transpose(1, 0, 2)
            .reshape(128, NMM * 128))
        per_core.append(dict(
            idx_lo=_pack_idxs(idx_streams[0][c]),
            idx_hi=_pack_idxs(idx_streams[1][c]) if T_half[1] else None,
            mask=mk,
        ))
    return sched, per_core


def _host_prep(N, ncores, adj_rows, adj_cols, adj_vals):
    S = N // ncores
    assert S * ncores == N
    NB = (S + 127) // 128
    CH_B = -(-NB // NCH_AG)          # blocks per AllGather chunk
    CH_R = CH_B * 128                # rows per chunk per core
    T2_ROWS = ncores * NCH_AG * CH_R  # chunk-major table2 size

    rows = adj_rows.astype(np.int64)
    cols = adj_cols.astype(np.int64)
    vals2 = (2.0 * adj_vals).astype(np.float32)

    core = rows // S
    dloc = rows - core * S

    # phase 2: remap source node (c, r) -> chunk-major table2 row
    c2 = cols // S
    r2 = cols - c2 * S
    k2 = r2 // CH_R
    col_remap = (k2 * ncores + c2) * CH_R + (r2 - k2 * CH_R)
    assert col_remap.max() < T2_ROWS <= 65536

    sched1, pc1 = _build_schedule(ncores, NB, core, dloc, cols, vals2, cols)
    sched2, pc2 = _build_schedule(ncores, NB, core, dloc, col_remap, vals2,
                                  cols)
    sched = dict(S=S, NB=NB, CH_B=CH_B, CH_R=CH_R, T2_ROWS=T2_ROWS,
                 p1=sched1, p2=sched2)
    per_core = [dict(p1=pc1[c], p2=pc2[c]) for c in range(ncores)]
    return sched, per_core


def _build_program(N, ncores, sched):
    S, NB = sched["S"], sched["NB"]
    CH_B, CH_R, T2_ROWS = sched["CH_B"], sched["CH_R"], sched["T2_ROWS"]

    nc = bacc.Bacc("TRN2", target_bir_lowering=False,
                   num_devices=(ncores if ncores > 1 else None))

    tab_d = nc.dram_tensor("tab", [N, D], BF16, kind="ExternalInput")
    xT_d = nc.dram_tensor("xT", [D, NB * 128], BF16, kind="ExternalInput")
    w_d = nc.dram_tensor("w", [D, D], F32, kind="ExternalInput")
    wT_d = nc.dram_tensor("wT", [D, D], F32, kind="ExternalInput")
    y_d = nc.dram_tensor("y", [S, D], F32, kind="ExternalOutput")

    cc_in = [nc.dram_tensor(f"cc_in{k}", [CH_R, D], BF16, kind="Internal")
             for k in range(NCH_AG)]
    cc_out = nc.dram_tensor("cc_out", [T2_ROWS, D], BF16, kind="Internal",
                            addr_space="Shared")

    # per-phase index streams + mask streams
    phase_io = []
    for p in (1, 2):
        ps = sched[f"p{p}"]
        T_lo, T_hi, NMM = ps["T_lo"], ps["T_hi"], ps["NMM"]
        ixlo_d = nc.dram_tensor(f"ixlo{p}", [128, T_lo * 8], I16,
                                kind="ExternalInput")
        ixhi_d = (nc.dram_tensor(f"ixhi{p}", [128, T_hi * 8], I16,
                                 kind="ExternalInput") if T_hi else None)
        mask_d = nc.dram_tensor(f"mask{p}", [128, NMM * 128], BF16,
                                kind="ExternalInput")
        ixlo_sb = nc.alloc_sbuf_tensor(f"ixlo{p}_sb", [128, T_lo * 8], I16)
        ixhi_sb = (nc.alloc_sbuf_tensor(f"ixhi{p}_sb", [128, T_hi * 8], I16)
                   if T_hi else None)
        phase_io.append(dict(ixlo_d=ixlo_d, ixhi_d=ixhi_d, mask_d=mask_d,
                             ixlo_sb=ixlo_sb, ixhi_sb=ixhi_sb))

    xT_sb = nc.alloc_sbuf_tensor("xT_sb", [D, NB * 128], BF16)
    w_sb = nc.alloc_sbuf_tensor("w_sb", [D, D], F32)
    wT_sb = nc.alloc_sbuf_tensor("wT_sb", [D, D], F32)
    w2_sb = nc.alloc_sbuf_tensor("w2_sb", [D, D], F32)
    w2bf_sb = nc.alloc_sbuf_tensor("w2bf_sb", [D, D], BF16)
    w3bf_sb = nc.alloc_sbuf_tensor("w3bf_sb", [D, D], BF16)
    v_sb = nc.alloc_sbuf_tensor("v_sb", [128, NB * 128], F32)

    def chunks(tot):
        out = []
        t0 = 0
        while t0 < tot:
            ct = min(CH_TILES, tot - t0)
            out.append((t0, ct))
            t0 += ct
        return out

    with tile.TileContext(nc) as tc:
        # first gather chunk's indices land first: tiny DMA up front
        io1, io2 = phase_io
        pre = min(CH_TILES * 8, io1["ixlo_sb"].shape[1])
        nc.sync.dma_start(io1["ixlo_sb"][:, :pre], io1["ixlo_d"][:, :pre])
        nc.sync.dma_start(io1["ixlo_sb"][:, pre:], io1["ixlo_d"][:, pre:])
        if io1["ixhi_sb"] is not None:
            nc.sync.dma_start(io1["ixhi_sb"][:], io1["ixhi_d"][:])
        nc.sync.dma_start(xT_sb[:], xT_d[:])
        nc.sync.dma_start(w_sb[:], w_d[:])
        nc.sync.dma_start(wT_sb[:], wT_d[:])
        nc.sync.dma_start(io2["ixlo_sb"][:], io2["ixlo_d"][:])
        if io2["ixhi_sb"] is not None:
            nc.sync.dma_start(io2["ixhi_sb"][:], io2["ixhi_d"][:])

        with (
            tc.tile_pool(name="wps", bufs=2, space="PSUM") as wps,
        ):
            w2_ps = wps.tile([D, D], F32, name="w2_ps")
            nc.tensor.matmul(w2_ps[:], wT_sb[:], w_sb[:], start=True, stop=True)
            nc.vector.tensor_copy(w2_sb[:], w2_ps[:])
            nc.vector.tensor_copy(w2bf_sb[:], w2_ps[:])
            w3_ps = wps.tile([D, D], F32, name="w3_ps")
            nc.tensor.matmul(w3_ps[:], wT_sb[:], w2_sb[:], start=True, stop=True)
            nc.vector.tensor_copy(w3bf_sb[:], w3_ps[:])

        def emit_spmm(phase, io, ps_sched, tab_lo_ap, tab_hi_ap,
                      per_block_tail, after_block):
            T_lo, T_hi = ps_sched["T_lo"], ps_sched["T_hi"]
            NMM = ps_sched["NMM"]
            block_mms = ps_sched["block_mms"]
            stream_chunks = (chunks(T_lo), chunks(T_hi))
            with (
                tc.tile_pool(name=f"g{phase}", bufs=5) as gpool,
                tc.tile_pool(name=f"m{phase}", bufs=4) as mpool,
                tc.tile_pool(name=f"ps{phase}", bufs=3, space="PSUM") as ppool,
                tc.tile_pool(name=f"tail{phase}", bufs=2, space="PSUM") as tpool,
                tc.tile_pool(name=f"sb{phase}", bufs=3) as spool,
            ):
                gbufs = {}
                mbufs = {}

                def ensure_chunk(h, ci):
                    k = (h, ci)
                    if k in gbufs:
                        return gbufs[k]
                    t0, ct = stream_chunks[h][ci]
                    n = ct * 128
                    g = gpool.tile([128, CH_TILES, 128], BF16,
                                   tag=f"g{h}", name=f"g{phase}_{h}_{ci}")
                    ix = (io["ixlo_sb"], io["ixhi_sb"])[h]
                    tab = (tab_lo_ap, tab_hi_ap)[h]
                    nc.gpsimd.dma_gather(
                        g[:, :ct, :], tab, ix[:, t0 * 8:(t0 + ct) * 8], n, n, D)
                    gbufs[k] = g
                    return g

                def ensure_mchunk(ci):
                    if ci in mbufs:
                        return mbufs[ci]
                    n = min(MCH, NMM - ci * MCH)
                    mt = mpool.tile([128, MCH * 128], BF16, tag="mk",
                                    name=f"mk{phase}_{ci}")
                    nc.sync.dma_start(
                        mt[:, :n * 128],
                        io["mask_d"][:, ci * MCH * 128:(ci * MCH + n) * 128])
                    mbufs[ci] = mt
                    return mt

                mm_ctr = [0]

                def next_mask():
                    m = mm_ctr[0]
                    mm_ctr[0] += 1
                    mt = ensure_mchunk(m // MCH)
                    off = m % MCH
                    return mt[:, off * 128:(off + 1) * 128]

                for b in range(NB):
                    mms = block_mms[b]
                    ps = ppool.tile([128, 128], F32, tag="ps", name=f"ps{phase}_{b}")
                    for j, (h, t, sl) in enumerate(mms):
                        g = ensure_chunk(h, t // CH_TILES)
                        tic = t % CH_TILES
                        msl = next_mask()
                        first, last = (j == 0), (j == len(mms) - 1)
                        if phase == 1:
                            nc.tensor.matmul(ps[:], g[:, tic, :], msl,
                                             start=first, stop=last)
                        else:
                            nc.tensor.matmul(ps[:], msl, g[:, tic, :],
                                             start=first, stop=last)
                    per_block_tail(b, ps, tpool, spool)
                    after_block(b)

        def tail1(b, ps, tpool, spool):
            rows = min(128, S - 128 * b)
            z1t = spool.tile([128, 128], BF16, tag="z1t", name=f"z1t_{b}")
            nc.scalar.copy(z1t[:], ps[:])                      # ACT [f,d] bf16
            t2_ps = tpool.tile([128, 128], F32, tag="t2ps", name=f"t2ps_{b}")
            nc.tensor.matmul(t2_ps[:], z1t[:], w3bf_sb[:], start=True, stop=True)
            u_ps = tpool.tile([128, 128], F32, tag="ups", name=f"ups_{b}")
            nc.tensor.matmul(u_ps[:], xT_sb[:, b * 128:(b + 1) * 128],
                             w2bf_sb[:], start=True, stop=True)
            t2t = spool.tile([128, 128], BF16, tag="t2t", name=f"t2t_{b}")
            nc.scalar.copy(t2t[:], t2_ps[:])                   # ACT f32->bf16
            nc.vector.tensor_tensor(v_sb[:, b * 128:(b + 1) * 128],
                                    u_ps[:], t2t[:], mybir.AluOpType.add)
            k = b // CH_B
            lo = b * 128 - k * CH_R
            nc.sync.dma_start(cc_in[k][lo:lo + rows, :], t2t[:rows, :])

        ag_done = [False] * NCH_AG

        def fire_ag(k):
            if ag_done[k]:
                return
            ag_done[k] = True
            if ncores > 1:
                nc.gpsimd.collective_compute(
                    "AllGather", mybir.AluOpType.bypass,
                    replica_groups=[list(range(ncores))],
                    ins=[cc_in[k][:]],
                    outs=[cc_out[k * ncores * CH_R:(k + 1) * ncores * CH_R, :]])
            else:
                nc.sync.dma_start(
                    cc_out[k * CH_R:(k + 1) * CH_R, :], cc_in[k][:])

        def after_block1(b):
            k = (b - AG_LOOKAHEAD) // CH_B
            if 0 <= k < NCH_AG and (b - AG_LOOKAHEAD) % CH_B == CH_B - 1:
                fire_ag(k)

        def tail2(b, ps, tpool, spool):
            rows = min(128, S - 128 * b)
            y = spool.tile([128, 128], F32, tag="y", name=f"y_{b}")
            nc.vector.tensor_tensor(y[:], ps[:], v_sb[:, b * 128:(b + 1) * 128],
                                    mybir.AluOpType.subtract)
            nc.sync.dma_start(y_d[b * 128:b * 128 + rows, :], y[:rows, :])

        hi_rows = N - SPLIT if N > SPLIT else 0
        emit_spmm(1, phase_io[0], sched["p1"], tab_d[0:min(SPLIT, N), :],
                  tab_d[SPLIT:N, :] if hi_rows else None, tail1, after_block1)
        for k in range(NCH_AG):
            fire_ag(k)

        t2_hi = T2_ROWS - SPLIT if T2_ROWS > SPLIT else 0
        emit_spmm(2, phase_io[1], sched["p2"], cc_out[0:min(SPLIT, T2_ROWS), :],
                  cc_out[SPLIT:T2_ROWS, :] if t2_hi else None, tail2,
                  lambda b: None)

    nc.compile()
    return nc


def _make_in_maps(N, ncores, sched, per_core, input_np, W_np):
    S, NB = sched["S"], sched["NB"]
    tab = input_np.astype(ml_dtypes.bfloat16)
    W = W_np.astype(np.float32)
    WT = np.ascontiguousarray(W.T)
    in_maps = []
    for c in range(ncores):
        xT = np.zeros((D, NB * 128), ml_dtypes.bfloat16)
        xT[:, :S] = tab[c * S:(c + 1) * S].T
        m = dict(tab=tab, xT=xT, w=W, wT=WT)
        for p in (1, 2):
            pc = per_core[c][f"p{p}"]
            m[f"ixlo{p}"] = pc["idx_lo"]
            m[f"mask{p}"] = pc["mask"]
            if sched[f"p{p}"]["T_hi"]:
                m[f"ixhi{p}"] = pc["idx_hi"]
        in_maps.append(m)
    return in_maps


_cache = {}


def _get_program(N, ncores, sched):
    key = (N, ncores) + tuple(
        (sched[p]["NMM"], sched[p]["T_lo"], sched[p]["T_hi"])
        for p in ("p1", "p2"))
    if key not in _cache:
        _cache[key] = _build_program(N, ncores, sched)
    return _cache[key]


def run(input, adj_rows, adj_cols, adj_vals, W, ncores=8, trace=False):
    N = input.shape[0]
    sched, per_core = _host_prep(N, ncores, adj_rows, adj_cols, adj_vals)
    nc = _get_program(N, ncores, sched)
    in_maps = _make_in_maps(N, ncores, sched, per_core, np.asarray(input),
                            np.asarray(W))
    res = run_bass_kernel_spmd(nc, in_maps, core_ids=list(range(ncores)),
                               trace=trace)
    y = np.concatenate([res.results[c]["y"] for c in range(ncores)], axis=0)
    return y[:N].astype(np.float32), res


def kernel(input, adj_rows, adj_cols, adj_vals, W):
    y, _ = run(np.asarray(input), np.asarray(adj_rows), np.asarray(adj_cols),
               np.asarray(adj_vals), np.asarray(W), ncores=8)
    return y


# revision 11
# speedup vs baseline: 1.0526x; 1.0526x over previous
"""ChebConvolution (K=4) Trainium2 kernel, 8-way sharded.

Math: with P = spmm(2*adj_vals) and right-multiplication by W commuting
with the (linear) sparse propagation, the reference collapses to

    Y = P(table2) - V,   table2 = Z1 @ W^3,  Z1 = P X,
                         V = Z1 @ W^3 + X @ W^2

Per core c (rows [c*S, (c+1)*S)):
  phase 1: SpMM Z1 rows via dma_gather from replicated bf16 X table +
           host-precomputed val-at-dest mask slabs streamed from HBM,
           accumulated in PSUM (feature-major) via mask matmuls,
           then t2 = Z1 @ W3 and V = t2 + X @ W2 per 128-row block.
  AllGather t2 shards -> full bf16 table2 in every core's HBM. The Q7
           SWDGE bubble is hidden: phase-2's first PREP_CH gather chunks
           are issued prepare_only (descriptor generation is data-
           independent) while tail1 drains + the collective runs, then one
           trigger_dma fires them once table2 lands.
  phase 2: SpMM P(table2) rows (node-major psum), subtract V, write Y shard.

Edges are sorted by dest; each 128-edge tile maps to an anchor dest block;
its [128, 128] mask slabs (up to 3 per tile for dests in [128*anchor,
128*anchor+384)) hold val[e] at [slot, dest_rel], built ON HOST and DMA-
streamed in consumption order. Q7 SWDGE descriptor generation (~8.4ns/idx,
hard num_idxs<=1024/call, single queue) is the serial bottleneck; mask DMA,
PE matmuls, tails and the collective hide under its shadow.

Edges are partitioned by dest core and split by source half (int16 gather
index limit); per-(core,half) streams are padded only at the end to a
cross-core-uniform tile count so one NEFF serves all 8 cores.
"""

import os
import sys

for _p in ("/opt/trn_rl_repo", "/root/.axon_site/_ro/trn_rl_repo"):
    if os.path.isdir(_p) and _p not in sys.path:
        sys.path.insert(0, _p)

import numpy as np
import ml_dtypes

import concourse.bacc as bacc
import concourse.mybir as mybir
import concourse.tile as tile
from concourse.bass_utils import run_bass_kernel_spmd

F32 = mybir.dt.float32
BF16 = mybir.dt.bfloat16
I16 = mybir.dt.int16

D = 128            # feature dim (in == out == 128)
SPLIT = 32768      # int16 gather index limit -> lo/hi table halves
CH_TILES = 8       # gather chunk: 1024 idx (HW dma_gather limit)
MCH = 8            # mask slabs per DMA chunk
PREP_CH = 8        # phase-2 chunks prepared (desc-gen) under the AllGather


def _pack_idxs(flat_idx):
    """int16 gather index layout: [128, n/16], idx j at [16k + j%16, j//16]."""
    n = len(flat_idx)
    assert n % 16 == 0
    arr = flat_idx.astype(np.int16).reshape(n // 16, 16).T  # [16, n/16]
    return np.tile(arr, (8, 1))


def _host_prep(N, ncores, adj_rows, adj_cols, adj_vals):
    """Sort/pad edges into per-core uniform tile streams + union schedule.

    Returns sched (cross-core constants incl. per-block mm lists) and
    per-core input arrays (gather indices + mask slab stream).
    """
    S = N // ncores
    NB = (S + 127) // 128
    rows = adj_rows.astype(np.int64)
    cols = adj_cols.astype(np.int64)
    vals2 = (2.0 * adj_vals).astype(np.float32)

    core = rows // S
    dloc = rows - core * S
    half = (cols >= SPLIT).astype(np.int64)

    ch_key = core * 2 + half
    cnt = np.bincount(ch_key, minlength=ncores * 2).reshape(ncores, 2)
    T_half = [max(int(-(-cnt[:, h].max() // 128)), 1) for h in (0, 1)]
    if N <= SPLIT:
        T_half[1] = 0

    order = np.lexsort((cols, dloc, half, core))
    k_s = ch_key[order]
    firsts = np.r_[0, np.flatnonzero(np.diff(k_s)) + 1]
    seg_of = np.cumsum(np.isin(np.arange(len(k_s)), firsts)) - 1
    rank = np.arange(len(k_s)) - firsts[seg_of]

    core_s, half_s = core[order], half[order]
    col_s, dloc_s, val_s = cols[order], dloc[order], vals2[order]
    tile_in_half = rank // 128

    anchors, straddles = [], []
    for h in range(2):
        T = max(T_half[h], 1)
        tmin = np.full(T, 1 << 30, np.int64)
        tmax = np.full(T, -1, np.int64)
        m = half_s == h
        if m.any():
            np.minimum.at(tmin, tile_in_half[m], dloc_s[m])
            np.maximum.at(tmax, tile_in_half[m], dloc_s[m])
        anchor = np.where(tmax >= 0, np.minimum(tmin // 128, NB - 1), 0)
        assert (tmax < anchor * 128 + 384).all(), "tile dest span exceeds 384"
        anchors.append(anchor)
        straddles.append((tmax >= (anchor + 1) * 128,
                          tmax >= (anchor + 2) * 128))

    block_mms = [[] for _ in range(NB)]
    for h in range(2):
        if T_half[h] == 0:
            continue
        for t in range(T_half[h]):
            b = int(anchors[h][t])
            block_mms[b].append((h, t, 0))
            if straddles[h][0][t]:
                block_mms[b + 1].append((h, t, 1))
            if straddles[h][1][t]:
                block_mms[b + 2].append((h, t, 2))
    assert all(block_mms[b] for b in range(NB)), "block with no matmuls"
    mm_ord = {}
    for b in range(NB):
        for key in block_mms[b]:
            mm_ord[key] = len(mm_ord)
    NMM = len(mm_ord)

    T_tot = T_half[0] + T_half[1]
    idx_streams = [np.zeros((ncores, max(T_half[h], 1) * 128), np.int64)
                   for h in range(2)]
    for h in range(2):
        m = half_s == h
        if not m.any():
            continue
        idx_streams[h][core_s[m], rank[m]] = col_s[m] - (SPLIT if h else 0)

    t0c = np.minimum(tile_in_half, max(T_half[0], 1) - 1)
    t1c = np.minimum(tile_in_half, max(T_half[1], 1) - 1)
    anchor_s = np.where(half_s == 0, anchors[0][t0c], anchors[1][t1c])
    rel = dloc_s - 128 * anchor_s
    assert ((rel >= 0) & (rel < 384)).all()

    lut = np.full((2, max(T_half[0], T_half[1], 1), 3), -1, np.int64)
    for (h, t, sl), o in mm_ord.items():
        lut[h, t, sl] = o
    slab_id = lut[half_s, tile_in_half, rel // 128]
    assert (slab_id >= 0).all()

    mask_all = np.zeros((ncores, NMM * 128 * 128), ml_dtypes.bfloat16)
    mpos = slab_id * (128 * 128) + (rank % 128) * 128 + (rel % 128)
    mask_all[core_s, mpos] = val_s.astype(ml_dtypes.bfloat16)

    sched = dict(S=S, NB=NB, T_lo=T_half[0], T_hi=T_half[1], NMM=NMM,
                 T_tot=T_tot, block_mms=block_mms)
    per_core = []
    for c in range(ncores):
        mk = np.ascontiguousarray(
            mask_all[c].reshape(NMM, 128, 128).transpose(1, 0, 2)
            .reshape(128, NMM * 128))
        per_core.append(dict(
            idx_lo=_pack_idxs(idx_streams[0][c]),
            idx_hi=_pack_idxs(idx_streams[1][c]) if T_half[1] else None,
            mask=mk,
        ))
    return sched, per_core


def _chunk_list(tot):
    out = []
    t0 = 0
    while t0 < tot:
        ct = min(CH_TILES, tot - t0)
        out.append((t0, ct))
        t0 += ct
    return out


def _chunk_use_order(sched):
    """First-use order of (half, gather-chunk) pairs over the block loop."""
    seen = []
    have = set()
    for b in range(sched["NB"]):
        for (h, t, sl) in sched["block_mms"][b]:
            k = (h, t // CH_TILES)
            if k not in have:
                have.add(k)
                seen.append(k)
    return seen


def _build_program(N, ncores, sched):
    S, NB = sched["S"], sched["NB"]
    T_lo, T_hi, NMM = sched["T_lo"], sched["T_hi"], sched["NMM"]
    block_mms = sched["block_mms"]

    nc = bacc.Bacc("TRN2", target_bir_lowering=False,
                   num_devices=(ncores if ncores > 1 else None))

    tab_d = nc.dram_tensor("tab", [N, D], BF16, kind="ExternalInput")
    xT_d = nc.dram_tensor("xT", [D, NB * 128], BF16, kind="ExternalInput")
    w_d = nc.dram_tensor("w", [D, D], F32, kind="ExternalInput")
    wT_d = nc.dram_tensor("wT", [D, D], F32, kind="ExternalInput")
    ixpre_d = nc.dram_tensor("ixpre", [128, CH_TILES * 8], I16,
                             kind="ExternalInput")
    ixlo_d = nc.dram_tensor("ixlo", [128, T_lo * 8], I16, kind="ExternalInput")
    if T_hi:
        ixhi_d = nc.dram_tensor("ixhi", [128, T_hi * 8], I16, kind="ExternalInput")
    mask_d = nc.dram_tensor("mask", [128, NMM * 128], BF16, kind="ExternalInput")
    y_d = nc.dram_tensor("y", [S, D], F32, kind="ExternalOutput")

    cc_in = nc.dram_tensor("cc_in", [S, D], BF16, kind="Internal")
    cc_out = nc.dram_tensor("cc_out", [N, D], BF16, kind="Internal",
                            addr_space="Shared")

    ixpre_sb = nc.alloc_sbuf_tensor("ixpre_sb", [128, CH_TILES * 8], I16)
    ixlo_sb = nc.alloc_sbuf_tensor("ixlo_sb", [128, T_lo * 8], I16)
    ixhi_sb = nc.alloc_sbuf_tensor("ixhi_sb", [128, T_hi * 8], I16) if T_hi else None
    xT_sb = nc.alloc_sbuf_tensor("xT_sb", [D, NB * 128], BF16)
    w_sb = nc.alloc_sbuf_tensor("w_sb", [D, D], F32)
    wT_sb = nc.alloc_sbuf_tensor("wT_sb", [D, D], F32)
    w2_sb = nc.alloc_sbuf_tensor("w2_sb", [D, D], F32)
    w2bf_sb = nc.alloc_sbuf_tensor("w2bf_sb", [D, D], BF16)
    w3bf_sb = nc.alloc_sbuf_tensor("w3bf_sb", [D, D], BF16)
    v_sb = nc.alloc_sbuf_tensor("v_sb", [128, NB * 128], F32)

    stream_chunks = (_chunk_list(T_lo), _chunk_list(T_hi))

    with tile.TileContext(nc) as tc:
        nc.sync.dma_start(ixpre_sb[:], ixpre_d[:])
        nc.sync.dma_start(ixlo_sb[:], ixlo_d[:])
        if T_hi:
            nc.sync.dma_start(ixhi_sb[:], ixhi_d[:])
        nc.sync.dma_start(xT_sb[:], xT_d[:])
        nc.sync.dma_start(w_sb[:], w_d[:])
        nc.sync.dma_start(wT_sb[:], wT_d[:])

        with (
            tc.tile_pool(name="wps", bufs=2, space="PSUM") as wps,
        ):
            w2_ps = wps.tile([D, D], F32, name="w2_ps")
            nc.tensor.matmul(w2_ps[:], wT_sb[:], w_sb[:], start=True, stop=True)
            nc.vector.tensor_copy(w2_sb[:], w2_ps[:])
            nc.vector.tensor_copy(w2bf_sb[:], w2_ps[:])
            w3_ps = wps.tile([D, D], F32, name="w3_ps")
            nc.tensor.matmul(w3_ps[:], wT_sb[:], w2_sb[:], start=True, stop=True)
            nc.vector.tensor_copy(w3bf_sb[:], w3_ps[:])

        def emit_spmm(phase, tab_lo_ap, tab_hi_ap, per_block_tail,
                      pre_gathered=None):
            with (
                tc.tile_pool(name=f"g{phase}", bufs=5) as gpool,
                tc.tile_pool(name=f"m{phase}", bufs=4) as mpool,
                tc.tile_pool(name=f"ps{phase}", bufs=3, space="PSUM") as ppool,
                tc.tile_pool(name=f"tail{phase}", bufs=2, space="PSUM") as tpool,
                tc.tile_pool(name=f"sb{phase}", bufs=3) as spool,
            ):
                gbufs = dict(pre_gathered or {})
                mbufs = {}

                def ensure_chunk(h, ci):
                    k = (h, ci)
                    if k in gbufs:
                        return gbufs[k]
                    t0, ct = stream_chunks[h][ci]
                    n = ct * 128
                    g = gpool.tile([128, CH_TILES, 128], BF16,
                                   tag=f"g{h}", name=f"g{phase}_{h}_{ci}")
                    if phase == 1 and h == 0 and ci == 0:
                        ix = ixpre_sb[:, 0:ct * 8]
                    else:
                        ix = (ixlo_sb, ixhi_sb)[h][:, t0 * 8:(t0 + ct) * 8]
                    tab = (tab_lo_ap, tab_hi_ap)[h]
                    nc.gpsimd.dma_gather(g[:, :ct, :], tab, ix, n, n, D)
                    gbufs[k] = g
                    return g

                def ensure_mchunk(ci):
                    if ci in mbufs:
                        return mbufs[ci]
                    n = min(MCH, NMM - ci * MCH)
                    mt = mpool.tile([128, MCH * 128], BF16, tag="mk",
                                    name=f"mk{phase}_{ci}")
                    nc.sync.dma_start(
                        mt[:, :n * 128],
                        mask_d[:, ci * MCH * 128:(ci * MCH + n) * 128])
                    mbufs[ci] = mt
                    return mt

                mm_ctr = [0]

                def next_mask():
                    m = mm_ctr[0]
                    mm_ctr[0] += 1
                    mt = ensure_mchunk(m // MCH)
                    off = m % MCH
                    return mt[:, off * 128:(off + 1) * 128]

                for b in range(NB):
                    mms = block_mms[b]
                    ps = ppool.tile([128, 128], F32, tag="ps", name=f"ps{phase}_{b}")
                    for j, (h, t, sl) in enumerate(mms):
                        g = ensure_chunk(h, t // CH_TILES)
                        tic = t % CH_TILES
                        msl = next_mask()
                        first, last = (j == 0), (j == len(mms) - 1)
                        if phase == 1:
                            nc.tensor.matmul(ps[:], g[:, tic, :], msl,
                                             start=first, stop=last)
                        else:
                            nc.tensor.matmul(ps[:], msl, g[:, tic, :],
                                             start=first, stop=last)
                    per_block_tail(b, ps, tpool, spool)

        def tail1(b, ps, tpool, spool):
            rows = min(128, S - 128 * b)
            z1t = spool.tile([128, 128], BF16, tag="z1t", name=f"z1t_{b}")
            nc.scalar.copy(z1t[:], ps[:])                      # ACT [f,d] bf16
            t2_ps = tpool.tile([128, 128], F32, tag="t2ps", name=f"t2ps_{b}")
            nc.tensor.matmul(t2_ps[:], z1t[:], w3bf_sb[:], start=True, stop=True)
            u_ps = tpool.tile([128, 128], F32, tag="ups", name=f"ups_{b}")
            nc.tensor.matmul(u_ps[:], xT_sb[:, b * 128:(b + 1) * 128],
                             w2bf_sb[:], start=True, stop=True)
            t2t = spool.tile([128, 128], BF16, tag="t2t", name=f"t2t_{b}")
            nc.scalar.copy(t2t[:], t2_ps[:])                   # ACT f32->bf16
            nc.vector.tensor_tensor(v_sb[:, b * 128:(b + 1) * 128],
                                    u_ps[:], t2t[:], mybir.AluOpType.add)
            nc.sync.dma_start(cc_in[b * 128:b * 128 + rows, :], t2t[:rows, :])

        def tail2(b, ps, tpool, spool):
            rows = min(128, S - 128 * b)
            y = spool.tile([128, 128], F32, tag="y", name=f"y_{b}")
            nc.vector.tensor_tensor(y[:], ps[:], v_sb[:, b * 128:(b + 1) * 128],
                                    mybir.AluOpType.subtract)
            nc.sync.dma_start(y_d[b * 128:b * 128 + rows, :], y[:rows, :])

        hi_rows = N - SPLIT if N > SPLIT else 0
        tab2_lo = cc_out[0:min(SPLIT, N), :]
        tab2_hi = cc_out[SPLIT:N, :] if hi_rows else None

        emit_spmm(1, tab_d[0:min(SPLIT, N), :],
                  tab_d[SPLIT:N, :] if hi_rows else None, tail1)

        if ncores > 1:
            nc.gpsimd.collective_compute(
                "AllGather", mybir.AluOpType.bypass,
                replica_groups=[list(range(ncores))],
                ins=[cc_in[:]], outs=[cc_out[:]])
        else:
            nc.sync.dma_start(cc_out[:], cc_in[:])

        emit_spmm(2, tab2_lo, tab2_hi, tail2)

    nc.compile()
    return nc


def _make_in_maps(N, ncores, sched, per_core, input_np, W_np):
    S, NB = sched["S"], sched["NB"]
    tab = input_np.astype(ml_dtypes.bfloat16)
    W = W_np.astype(np.float32)
    WT = np.ascontiguousarray(W.T)
    in_maps = []
    for c in range(ncores):
        xT = np.zeros((D, NB * 128), ml_dtypes.bfloat16)
        xT[:, :S] = tab[c * S:(c + 1) * S].T
        m = dict(tab=tab, xT=xT, w=W, wT=WT,
                 ixpre=np.ascontiguousarray(
                     per_core[c]["idx_lo"][:, :CH_TILES * 8]),
                 ixlo=per_core[c]["idx_lo"],
                 mask=per_core[c]["mask"])
        if sched["T_hi"]:
            m["ixhi"] = per_core[c]["idx_hi"]
        in_maps.append(m)
    return in_maps


_cache = {}


def _get_program(N, ncores, sched):
    key = (N, ncores, sched["NMM"], sched["T_lo"], sched["T_hi"])
    if key not in _cache:
        _cache[key] = _build_program(N, ncores, sched)
    return _cache[key]


def run(input, adj_rows, adj_cols, adj_vals, W, ncores=8, trace=False):
    N = input.shape[0]
    sched, per_core = _host_prep(N, ncores, adj_rows, adj_cols, adj_vals)
    nc = _get_program(N, ncores, sched)
    in_maps = _make_in_maps(N, ncores, sched, per_core, np.asarray(input),
                            np.asarray(W))
    res = run_bass_kernel_spmd(nc, in_maps, core_ids=list(range(ncores)),
                               trace=trace)
    y = np.concatenate([res.results[c]["y"] for c in range(ncores)], axis=0)
    return y[:N].astype(np.float32), res


def kernel(input, adj_rows, adj_cols, adj_vals, W):
    y, _ = run(np.asarray(input), np.asarray(adj_rows), np.asarray(adj_cols),
               np.asarray(adj_vals), np.asarray(W), ncores=8)
    return y


# revision 15
# speedup vs baseline: 2.3907x; 2.2713x over previous
"""ChebConvolution (K=4) Trainium2 kernel, 8-way sharded.

Math: with P = spmm(2*adj_vals) and right-multiplication by W commuting
with the (linear) sparse propagation, the reference collapses to

    Y = P(table2) - V,   table2 = Z1 @ W^3,  Z1 = P X,
                         V = Z1 @ W^3 + X @ W^2

Per core c (rows [c*S, (c+1)*S)):
  phase 1: SpMM Z1 rows via dma_gather from replicated bf16 X table +
           host-precomputed val-at-dest mask slabs streamed from HBM,
           accumulated in PSUM (feature-major) via mask matmuls,
           then t2 = Z1 @ W3 and V = t2 + X @ W2 per 128-row block.
  AllGather t2 shards -> full bf16 table2 in every core's HBM.
  phase 2: SpMM P(table2) rows (node-major psum), subtract V, write Y shard.

  (Tried and rejected: chunked AllGather — collective per-chunk overhead
  ~4x total network time AND ring-drain contention slows every gather ~9%;
  prepare_only phase-2 desc-gen under the collective — tile keeps the
  table-read dep on the prep, so preps serialize after the AllGather, and
  pre-collective preps WAR-deadlock.)

Edges are sorted by dest; each 128-edge tile maps to an anchor dest block;
its [128, 128] mask slabs (up to 3 per tile for dests in [128*anchor,
128*anchor+384)) hold val[e] at [slot, dest_rel], built ON HOST and DMA-
streamed in consumption order. Q7 SWDGE descriptor generation (~8.4ns/idx,
hard num_idxs<=1024/call, single queue) is the serial bottleneck; mask DMA,
PE matmuls, tails and the collective hide under its shadow.

Edges are partitioned by dest core and split by source half (int16 gather
index limit); per-(core,half) streams are padded only at the end to a
cross-core-uniform tile count so one NEFF serves all 8 cores.
"""

import os
import sys

for _p in ("/opt/trn_rl_repo", "/root/.axon_site/_ro/trn_rl_repo"):
    if os.path.isdir(_p) and _p not in sys.path:
        sys.path.insert(0, _p)

import numpy as np
import ml_dtypes

import concourse.bacc as bacc
import concourse.mybir as mybir
import concourse.tile as tile
from concourse.bass_utils import run_bass_kernel_spmd

F32 = mybir.dt.float32
BF16 = mybir.dt.bfloat16
I16 = mybir.dt.int16

D = 128            # feature dim (in == out == 128)
SPLIT = 32768      # int16 gather index limit -> lo/hi table halves
CH_TILES = 8       # gather chunk: 1024 idx (HW dma_gather limit)
MCH = 8            # mask slabs per DMA chunk
PREP_CH = 8        # phase-2 chunks prepared (desc-gen) under the AllGather


def _pack_idxs(flat_idx):
    """int16 gather index layout: [128, n/16], idx j at [16k + j%16, j//16]."""
    n = len(flat_idx)
    assert n % 16 == 0
    arr = flat_idx.astype(np.int16).reshape(n // 16, 16).T  # [16, n/16]
    return np.tile(arr, (8, 1))


def _host_prep(N, ncores, adj_rows, adj_cols, adj_vals):
    """Sort/pad edges into per-core uniform tile streams + union schedule.

    Returns sched (cross-core constants incl. per-block mm lists) and
    per-core input arrays (gather indices + mask slab stream).
    """
    S = N // ncores
    NB = (S + 127) // 128
    rows = adj_rows.astype(np.int64)
    cols = adj_cols.astype(np.int64)
    vals2 = (2.0 * adj_vals).astype(np.float32)

    core = rows // S
    dloc = rows - core * S
    half = (cols >= SPLIT).astype(np.int64)

    ch_key = core * 2 + half
    cnt = np.bincount(ch_key, minlength=ncores * 2).reshape(ncores, 2)
    T_half = [max(int(-(-cnt[:, h].max() // 128)), 1) for h in (0, 1)]
    if N <= SPLIT:
        T_half[1] = 0

    order = np.lexsort((cols, dloc, half, core))
    k_s = ch_key[order]
    firsts = np.r_[0, np.flatnonzero(np.diff(k_s)) + 1]
    seg_of = np.cumsum(np.isin(np.arange(len(k_s)), firsts)) - 1
    rank = np.arange(len(k_s)) - firsts[seg_of]

    core_s, half_s = core[order], half[order]
    col_s, dloc_s, val_s = cols[order], dloc[order], vals2[order]
    tile_in_half = rank // 128

    anchors, straddles = [], []
    for h in range(2):
        T = max(T_half[h], 1)
        tmin = np.full(T, 1 << 30, np.int64)
        tmax = np.full(T, -1, np.int64)
        m = half_s == h
        if m.any():
            np.minimum.at(tmin, tile_in_half[m], dloc_s[m])
            np.maximum.at(tmax, tile_in_half[m], dloc_s[m])
        anchor = np.where(tmax >= 0, np.minimum(tmin // 128, NB - 1), 0)
        assert (tmax < anchor * 128 + 384).all(), "tile dest span exceeds 384"
        anchors.append(anchor)
        straddles.append((tmax >= (anchor + 1) * 128,
                          tmax >= (anchor + 2) * 128))

    block_mms = [[] for _ in range(NB)]
    for h in range(2):
        if T_half[h] == 0:
            continue
        for t in range(T_half[h]):
            b = int(anchors[h][t])
            block_mms[b].append((h, t, 0))
            if straddles[h][0][t]:
                block_mms[b + 1].append((h, t, 1))
            if straddles[h][1][t]:
                block_mms[b + 2].append((h, t, 2))
    assert all(block_mms[b] for b in range(NB)), "block with no matmuls"
    mm_ord = {}
    for b in range(NB):
        for key in block_mms[b]:
            mm_ord[key] = len(mm_ord)
    NMM = len(mm_ord)

    T_tot = T_half[0] + T_half[1]
    idx_streams = [np.zeros((ncores, max(T_half[h], 1) * 128), np.int64)
                   for h in range(2)]
    for h in range(2):
        m = half_s == h
        if not m.any():
            continue
        idx_streams[h][core_s[m], rank[m]] = col_s[m] - (SPLIT if h else 0)

    t0c = np.minimum(tile_in_half, max(T_half[0], 1) - 1)
    t1c = np.minimum(tile_in_half, max(T_half[1], 1) - 1)
    anchor_s = np.where(half_s == 0, anchors[0][t0c], anchors[1][t1c])
    rel = dloc_s - 128 * anchor_s
    assert ((rel >= 0) & (rel < 384)).all()

    lut = np.full((2, max(T_half[0], T_half[1], 1), 3), -1, np.int64)
    for (h, t, sl), o in mm_ord.items():
        lut[h, t, sl] = o
    slab_id = lut[half_s, tile_in_half, rel // 128]
    assert (slab_id >= 0).all()

    mask_all = np.zeros((ncores, NMM * 128 * 128), ml_dtypes.bfloat16)
    mpos = slab_id * (128 * 128) + (rank % 128) * 128 + (rel % 128)
    mask_all[core_s, mpos] = val_s.astype(ml_dtypes.bfloat16)

    sched = dict(S=S, NB=NB, T_lo=T_half[0], T_hi=T_half[1], NMM=NMM,
                 T_tot=T_tot, block_mms=block_mms)
    per_core = []
    for c in range(ncores):
        mk = np.ascontiguousarray(
            mask_all[c].reshape(NMM, 128, 128).transpose(1, 0, 2)
            .reshape(128, NMM * 128))
        per_core.append(dict(
            idx_lo=_pack_idxs(idx_streams[0][c]),
            idx_hi=_pack_idxs(idx_streams[1][c]) if T_half[1] else None,
            mask=mk,
        ))
    return sched, per_core


def _chunk_list(tot):
    out = []
    t0 = 0
    while t0 < tot:
        ct = min(CH_TILES, tot - t0)
        out.append((t0, ct))
        t0 += ct
    return out


def _chunk_use_order(sched):
    """First-use order of (half, gather-chunk) pairs over the block loop."""
    seen = []
    have = set()
    for b in range(sched["NB"]):
        for (h, t, sl) in sched["block_mms"][b]:
            k = (h, t // CH_TILES)
            if k not in have:
                have.add(k)
                seen.append(k)
    return seen


def _build_program(N, ncores, sched):
    S, NB = sched["S"], sched["NB"]
    T_lo, T_hi, NMM = sched["T_lo"], sched["T_hi"], sched["NMM"]
    block_mms = sched["block_mms"]

    nc = bacc.Bacc("TRN2", target_bir_lowering=False,
                   num_devices=(ncores if ncores > 1 else None),
                   num_swdge_queues=4)

    tab_d = nc.dram_tensor("tab", [N, D], BF16, kind="ExternalInput")
    xT_d = nc.dram_tensor("xT", [D, NB * 128], BF16, kind="ExternalInput")
    w_d = nc.dram_tensor("w", [D, D], F32, kind="ExternalInput")
    wT_d = nc.dram_tensor("wT", [D, D], F32, kind="ExternalInput")
    ixpre_d = nc.dram_tensor("ixpre", [128, CH_TILES * 8], I16,
                             kind="ExternalInput")
    ixlo_d = nc.dram_tensor("ixlo", [128, T_lo * 8], I16, kind="ExternalInput")
    if T_hi:
        ixhi_d = nc.dram_tensor("ixhi", [128, T_hi * 8], I16, kind="ExternalInput")
    mask_d = nc.dram_tensor("mask", [128, NMM * 128], BF16, kind="ExternalInput")
    y_d = nc.dram_tensor("y", [S, D], F32, kind="ExternalOutput")

    cc_in = nc.dram_tensor("cc_in", [S, D], BF16, kind="Internal")
    cc_out = nc.dram_tensor("cc_out", [N, D], BF16, kind="Internal",
                            addr_space="Shared")

    ixpre_sb = nc.alloc_sbuf_tensor("ixpre_sb", [128, CH_TILES * 8], I16)
    ixlo_sb = nc.alloc_sbuf_tensor("ixlo_sb", [128, T_lo * 8], I16)
    ixhi_sb = nc.alloc_sbuf_tensor("ixhi_sb", [128, T_hi * 8], I16) if T_hi else None
    xT_sb = nc.alloc_sbuf_tensor("xT_sb", [D, NB * 128], BF16)
    w_sb = nc.alloc_sbuf_tensor("w_sb", [D, D], F32)
    wT_sb = nc.alloc_sbuf_tensor("wT_sb", [D, D], F32)
    w2_sb = nc.alloc_sbuf_tensor("w2_sb", [D, D], F32)
    w2bf_sb = nc.alloc_sbuf_tensor("w2bf_sb", [D, D], BF16)
    w3bf_sb = nc.alloc_sbuf_tensor("w3bf_sb", [D, D], BF16)
    v_sb = nc.alloc_sbuf_tensor("v_sb", [128, NB * 128], F32)

    stream_chunks = (_chunk_list(T_lo), _chunk_list(T_hi))

    with tile.TileContext(nc) as tc:
        nc.sync.dma_start(ixpre_sb[:], ixpre_d[:])
        nc.sync.dma_start(ixlo_sb[:], ixlo_d[:])
        if T_hi:
            nc.sync.dma_start(ixhi_sb[:], ixhi_d[:])
        nc.sync.dma_start(xT_sb[:], xT_d[:])
        nc.sync.dma_start(w_sb[:], w_d[:])
        nc.sync.dma_start(wT_sb[:], wT_d[:])

        with (
            tc.tile_pool(name="wps", bufs=2, space="PSUM") as wps,
        ):
            w2_ps = wps.tile([D, D], F32, name="w2_ps")
            nc.tensor.matmul(w2_ps[:], wT_sb[:], w_sb[:], start=True, stop=True)
            nc.vector.tensor_copy(w2_sb[:], w2_ps[:])
            nc.vector.tensor_copy(w2bf_sb[:], w2_ps[:])
            w3_ps = wps.tile([D, D], F32, name="w3_ps")
            nc.tensor.matmul(w3_ps[:], wT_sb[:], w2_sb[:], start=True, stop=True)
            nc.vector.tensor_copy(w3bf_sb[:], w3_ps[:])

        qctr = [0]  # rotate gathers over SWDGE queues: ~2x desc-gen/drain overlap

        def emit_spmm(phase, tab_lo_ap, tab_hi_ap, per_block_tail,
                      pre_gathered=None):
            with (
                tc.tile_pool(name=f"g{phase}", bufs=5) as gpool,
                tc.tile_pool(name=f"m{phase}", bufs=4) as mpool,
                tc.tile_pool(name=f"ps{phase}", bufs=3, space="PSUM") as ppool,
                tc.tile_pool(name=f"tail{phase}", bufs=2, space="PSUM") as tpool,
                tc.tile_pool(name=f"sb{phase}", bufs=3) as spool,
            ):
                gbufs = dict(pre_gathered or {})
                mbufs = {}

                def ensure_chunk(h, ci):
                    k = (h, ci)
                    if k in gbufs:
                        return gbufs[k]
                    t0, ct = stream_chunks[h][ci]
                    n = ct * 128
                    g = gpool.tile([128, CH_TILES, 128], BF16,
                                   tag=f"g{h}", name=f"g{phase}_{h}_{ci}")
                    if phase == 1 and h == 0 and ci == 0:
                        ix = ixpre_sb[:, 0:ct * 8]
                    else:
                        ix = (ixlo_sb, ixhi_sb)[h][:, t0 * 8:(t0 + ct) * 8]
                    tab = (tab_lo_ap, tab_hi_ap)[h]
                    nc.gpsimd.dma_gather(g[:, :ct, :], tab, ix, n, n, D,
                                         queue_num=qctr[0] % 4)
                    qctr[0] += 1
                    gbufs[k] = g
                    return g

                def ensure_mchunk(ci):
                    if ci in mbufs:
                        return mbufs[ci]
                    n = min(MCH, NMM - ci * MCH)
                    mt = mpool.tile([128, MCH * 128], BF16, tag="mk",
                                    name=f"mk{phase}_{ci}")
                    nc.sync.dma_start(
                        mt[:, :n * 128],
                        mask_d[:, ci * MCH * 128:(ci * MCH + n) * 128])
                    mbufs[ci] = mt
                    return mt

                mm_ctr = [0]

                def next_mask():
                    m = mm_ctr[0]
                    mm_ctr[0] += 1
                    mt = ensure_mchunk(m // MCH)
                    off = m % MCH
                    return mt[:, off * 128:(off + 1) * 128]

                for b in range(NB):
                    mms = block_mms[b]
                    ps = ppool.tile([128, 128], F32, tag="ps", name=f"ps{phase}_{b}")
                    for j, (h, t, sl) in enumerate(mms):
                        g = ensure_chunk(h, t // CH_TILES)
                        tic = t % CH_TILES
                        msl = next_mask()
                        first, last = (j == 0), (j == len(mms) - 1)
                        if phase == 1:
                            nc.tensor.matmul(ps[:], g[:, tic, :], msl,
                                             start=first, stop=last)
                        else:
                            nc.tensor.matmul(ps[:], msl, g[:, tic, :],
                                             start=first, stop=last)
                    per_block_tail(b, ps, tpool, spool)

        def tail1(b, ps, tpool, spool):
            rows = min(128, S - 128 * b)
            z1t = spool.tile([128, 128], BF16, tag="z1t", name=f"z1t_{b}")
            nc.scalar.copy(z1t[:], ps[:])                      # ACT [f,d] bf16
            t2_ps = tpool.tile([128, 128], F32, tag="t2ps", name=f"t2ps_{b}")
            nc.tensor.matmul(t2_ps[:], z1t[:], w3bf_sb[:], start=True, stop=True)
            u_ps = tpool.tile([128, 128], F32, tag="ups", name=f"ups_{b}")
            nc.tensor.matmul(u_ps[:], xT_sb[:, b * 128:(b + 1) * 128],
                             w2bf_sb[:], start=True, stop=True)
            t2t = spool.tile([128, 128], BF16, tag="t2t", name=f"t2t_{b}")
            nc.scalar.copy(t2t[:], t2_ps[:])                   # ACT f32->bf16
            nc.vector.tensor_tensor(v_sb[:, b * 128:(b + 1) * 128],
                                    u_ps[:], t2t[:], mybir.AluOpType.add)
            nc.sync.dma_start(cc_in[b * 128:b * 128 + rows, :], t2t[:rows, :])

        def tail2(b, ps, tpool, spool):
            rows = min(128, S - 128 * b)
            y = spool.tile([128, 128], F32, tag="y", name=f"y_{b}")
            nc.vector.tensor_tensor(y[:], ps[:], v_sb[:, b * 128:(b + 1) * 128],
                                    mybir.AluOpType.subtract)
            nc.sync.dma_start(y_d[b * 128:b * 128 + rows, :], y[:rows, :])

        hi_rows = N - SPLIT if N > SPLIT else 0
        tab2_lo = cc_out[0:min(SPLIT, N), :]
        tab2_hi = cc_out[SPLIT:N, :] if hi_rows else None

        emit_spmm(1, tab_d[0:min(SPLIT, N), :],
                  tab_d[SPLIT:N, :] if hi_rows else None, tail1)

        if ncores > 1:
            nc.gpsimd.collective_compute(
                "AllGather", mybir.AluOpType.bypass,
                replica_groups=[list(range(ncores))],
                ins=[cc_in[:]], outs=[cc_out[:]])
        else:
            nc.sync.dma_start(cc_out[:], cc_in[:])

        emit_spmm(2, tab2_lo, tab2_hi, tail2)

    nc.compile()
    return nc


def _make_in_maps(N, ncores, sched, per_core, input_np, W_np):
    S, NB = sched["S"], sched["NB"]
    tab = input_np.astype(ml_dtypes.bfloat16)
    W = W_np.astype(np.float32)
    WT = np.ascontiguousarray(W.T)
    in_maps = []
    for c in range(ncores):
        xT = np.zeros((D, NB * 128), ml_dtypes.bfloat16)
        xT[:, :S] = tab[c * S:(c + 1) * S].T
        m = dict(tab=tab, xT=xT, w=W, wT=WT,
                 ixpre=np.ascontiguousarray(
                     per_core[c]["idx_lo"][:, :CH_TILES * 8]),
                 ixlo=per_core[c]["idx_lo"],
                 mask=per_core[c]["mask"])
        if sched["T_hi"]:
            m["ixhi"] = per_core[c]["idx_hi"]
        in_maps.append(m)
    return in_maps


_cache = {}


def _get_program(N, ncores, sched):
    key = (N, ncores, sched["NMM"], sched["T_lo"], sched["T_hi"])
    if key not in _cache:
        _cache[key] = _build_program(N, ncores, sched)
    return _cache[key]


def run(input, adj_rows, adj_cols, adj_vals, W, ncores=8, trace=False):
    N = input.shape[0]
    sched, per_core = _host_prep(N, ncores, adj_rows, adj_cols, adj_vals)
    nc = _get_program(N, ncores, sched)
    in_maps = _make_in_maps(N, ncores, sched, per_core, np.asarray(input),
                            np.asarray(W))
    res = run_bass_kernel_spmd(nc, in_maps, core_ids=list(range(ncores)),
                               trace=trace)
    y = np.concatenate([res.results[c]["y"] for c in range(ncores)], axis=0)
    return y[:N].astype(np.float32), res


def kernel(input, adj_rows, adj_cols, adj_vals, W):
    y, _ = run(np.asarray(input), np.asarray(adj_rows), np.asarray(adj_cols),
               np.asarray(adj_vals), np.asarray(W), ncores=8)
    return y


# revision 16
# speedup vs baseline: 2.5068x; 1.0485x over previous
"""ChebConvolution (K=4) Trainium2 kernel, 8-way sharded.

Math: with P = spmm(2*adj_vals) and right-multiplication by W commuting
with the (linear) sparse propagation, the reference collapses to

    Y = P(table2) - V,   table2 = Z1 @ W^3,  Z1 = P X,
                         V = Z1 @ W^3 + X @ W^2

Per core c (rows [c*S, (c+1)*S)):
  phase 1: SpMM Z1 rows via dma_gather from replicated bf16 X table +
           host-precomputed val-at-dest mask slabs streamed from HBM,
           accumulated in PSUM (feature-major) via mask matmuls,
           then t2 = Z1 @ W3 and V = t2 + X @ W2 per 128-row block.
  AllGather t2 shards -> full bf16 table2 in every core's HBM.
  phase 2: SpMM P(table2) rows (node-major psum), subtract V, write Y shard.

  (Tried and rejected: chunked AllGather — collective per-chunk overhead
  ~4x total network time AND ring-drain contention slows every gather ~9%;
  prepare_only phase-2 desc-gen under the collective — tile keeps the
  table-read dep on the prep, so preps serialize after the AllGather, and
  pre-collective preps WAR-deadlock.)

Edges are sorted by dest; each 128-edge tile maps to an anchor dest block;
its [128, 128] mask slabs (up to 3 per tile for dests in [128*anchor,
128*anchor+384)) hold val[e] at [slot, dest_rel], built ON HOST and DMA-
streamed in consumption order. Q7 SWDGE descriptor generation (~8.4ns/idx,
hard num_idxs<=1024/call, single queue) is the serial bottleneck; mask DMA,
PE matmuls, tails and the collective hide under its shadow.

Edges are partitioned by dest core and split by source half (int16 gather
index limit); per-(core,half) streams are padded only at the end to a
cross-core-uniform tile count so one NEFF serves all 8 cores.
"""

import os
import sys

for _p in ("/opt/trn_rl_repo", "/root/.axon_site/_ro/trn_rl_repo"):
    if os.path.isdir(_p) and _p not in sys.path:
        sys.path.insert(0, _p)

import numpy as np
import ml_dtypes

import concourse.bacc as bacc
import concourse.mybir as mybir
import concourse.tile as tile
from concourse.bass_utils import run_bass_kernel_spmd

F32 = mybir.dt.float32
BF16 = mybir.dt.bfloat16
I16 = mybir.dt.int16

D = 128            # feature dim (in == out == 128)
SPLIT = 32768      # int16 gather index limit -> lo/hi table halves
CH_TILES = 8       # gather chunk: 1024 idx (HW dma_gather limit)
MCH = 8            # mask slabs per DMA chunk
PREP_CH = 8        # phase-2 chunks prepared (desc-gen) under the AllGather


def _pack_idxs(flat_idx):
    """int16 gather index layout: [128, n/16], idx j at [16k + j%16, j//16]."""
    n = len(flat_idx)
    assert n % 16 == 0
    arr = flat_idx.astype(np.int16).reshape(n // 16, 16).T  # [16, n/16]
    return np.tile(arr, (8, 1))


def _host_prep(N, ncores, adj_rows, adj_cols, adj_vals):
    """Sort/pad edges into per-core uniform tile streams + union schedule.

    Returns sched (cross-core constants incl. per-block mm lists) and
    per-core input arrays (gather indices + mask slab stream).
    """
    S = N // ncores
    NB = (S + 127) // 128
    rows = adj_rows.astype(np.int64)
    cols = adj_cols.astype(np.int64)
    vals2 = (2.0 * adj_vals).astype(np.float32)

    core = rows // S
    dloc = rows - core * S
    half = (cols >= SPLIT).astype(np.int64)

    ch_key = core * 2 + half
    cnt = np.bincount(ch_key, minlength=ncores * 2).reshape(ncores, 2)
    T_half = [max(int(-(-cnt[:, h].max() // 128)), 1) for h in (0, 1)]
    if N <= SPLIT:
        T_half[1] = 0

    order = np.lexsort((cols, dloc, half, core))
    k_s = ch_key[order]
    firsts = np.r_[0, np.flatnonzero(np.diff(k_s)) + 1]
    seg_of = np.cumsum(np.isin(np.arange(len(k_s)), firsts)) - 1
    rank = np.arange(len(k_s)) - firsts[seg_of]

    core_s, half_s = core[order], half[order]
    col_s, dloc_s, val_s = cols[order], dloc[order], vals2[order]
    tile_in_half = rank // 128

    anchors, straddles = [], []
    for h in range(2):
        T = max(T_half[h], 1)
        tmin = np.full(T, 1 << 30, np.int64)
        tmax = np.full(T, -1, np.int64)
        m = half_s == h
        if m.any():
            np.minimum.at(tmin, tile_in_half[m], dloc_s[m])
            np.maximum.at(tmax, tile_in_half[m], dloc_s[m])
        anchor = np.where(tmax >= 0, np.minimum(tmin // 128, NB - 1), 0)
        assert (tmax < anchor * 128 + 384).all(), "tile dest span exceeds 384"
        anchors.append(anchor)
        straddles.append((tmax >= (anchor + 1) * 128,
                          tmax >= (anchor + 2) * 128))

    block_mms = [[] for _ in range(NB)]
    for h in range(2):
        if T_half[h] == 0:
            continue
        for t in range(T_half[h]):
            b = int(anchors[h][t])
            block_mms[b].append((h, t, 0))
            if straddles[h][0][t]:
                block_mms[b + 1].append((h, t, 1))
            if straddles[h][1][t]:
                block_mms[b + 2].append((h, t, 2))
    assert all(block_mms[b] for b in range(NB)), "block with no matmuls"
    mm_ord = {}
    for b in range(NB):
        for key in block_mms[b]:
            mm_ord[key] = len(mm_ord)
    NMM = len(mm_ord)

    T_tot = T_half[0] + T_half[1]
    idx_streams = [np.zeros((ncores, max(T_half[h], 1) * 128), np.int64)
                   for h in range(2)]
    for h in range(2):
        m = half_s == h
        if not m.any():
            continue
        idx_streams[h][core_s[m], rank[m]] = col_s[m] - (SPLIT if h else 0)

    t0c = np.minimum(tile_in_half, max(T_half[0], 1) - 1)
    t1c = np.minimum(tile_in_half, max(T_half[1], 1) - 1)
    anchor_s = np.where(half_s == 0, anchors[0][t0c], anchors[1][t1c])
    rel = dloc_s - 128 * anchor_s
    assert ((rel >= 0) & (rel < 384)).all()

    lut = np.full((2, max(T_half[0], T_half[1], 1), 3), -1, np.int64)
    for (h, t, sl), o in mm_ord.items():
        lut[h, t, sl] = o
    slab_id = lut[half_s, tile_in_half, rel // 128]
    assert (slab_id >= 0).all()

    mask_all = np.zeros((ncores, NMM * 128 * 128), ml_dtypes.bfloat16)
    mpos = slab_id * (128 * 128) + (rank % 128) * 128 + (rel % 128)
    mask_all[core_s, mpos] = val_s.astype(ml_dtypes.bfloat16)

    sched = dict(S=S, NB=NB, T_lo=T_half[0], T_hi=T_half[1], NMM=NMM,
                 T_tot=T_tot, block_mms=block_mms)
    per_core = []
    for c in range(ncores):
        mk = np.ascontiguousarray(
            mask_all[c].reshape(NMM, 128, 128).transpose(1, 0, 2)
            .reshape(128, NMM * 128))
        per_core.append(dict(
            idx_lo=_pack_idxs(idx_streams[0][c]),
            idx_hi=_pack_idxs(idx_streams[1][c]) if T_half[1] else None,
            mask=mk,
        ))
    return sched, per_core


def _chunk_list(tot):
    out = []
    t0 = 0
    while t0 < tot:
        ct = min(CH_TILES, tot - t0)
        out.append((t0, ct))
        t0 += ct
    return out


def _chunk_use_order(sched):
    """First-use order of (half, gather-chunk) pairs over the block loop."""
    seen = []
    have = set()
    for b in range(sched["NB"]):
        for (h, t, sl) in sched["block_mms"][b]:
            k = (h, t // CH_TILES)
            if k not in have:
                have.add(k)
                seen.append(k)
    return seen


def _build_program(N, ncores, sched):
    S, NB = sched["S"], sched["NB"]
    T_lo, T_hi, NMM = sched["T_lo"], sched["T_hi"], sched["NMM"]
    block_mms = sched["block_mms"]

    nc = bacc.Bacc("TRN2", target_bir_lowering=False,
                   num_devices=(ncores if ncores > 1 else None),
                   num_swdge_queues=4)

    tab_d = nc.dram_tensor("tab", [N, D], BF16, kind="ExternalInput")
    xT_d = nc.dram_tensor("xT", [D, NB * 128], BF16, kind="ExternalInput")
    w_d = nc.dram_tensor("w", [D, D], F32, kind="ExternalInput")
    wT_d = nc.dram_tensor("wT", [D, D], F32, kind="ExternalInput")
    ixpre_d = nc.dram_tensor("ixpre", [128, CH_TILES * 8], I16,
                             kind="ExternalInput")
    ixlo_d = nc.dram_tensor("ixlo", [128, T_lo * 8], I16, kind="ExternalInput")
    if T_hi:
        ixhi_d = nc.dram_tensor("ixhi", [128, T_hi * 8], I16, kind="ExternalInput")
    mask_d = nc.dram_tensor("mask", [128, NMM * 128], BF16, kind="ExternalInput")
    y_d = nc.dram_tensor("y", [S, D], F32, kind="ExternalOutput")

    cc_in = nc.dram_tensor("cc_in", [S, D], BF16, kind="Internal")
    cc_out = nc.dram_tensor("cc_out", [N, D], BF16, kind="Internal",
                            addr_space="Shared")

    ixpre_sb = nc.alloc_sbuf_tensor("ixpre_sb", [128, CH_TILES * 8], I16)
    ixlo_sb = nc.alloc_sbuf_tensor("ixlo_sb", [128, T_lo * 8], I16)
    ixhi_sb = nc.alloc_sbuf_tensor("ixhi_sb", [128, T_hi * 8], I16) if T_hi else None
    xT_sb = nc.alloc_sbuf_tensor("xT_sb", [D, NB * 128], BF16)
    w_sb = nc.alloc_sbuf_tensor("w_sb", [D, D], F32)
    wT_sb = nc.alloc_sbuf_tensor("wT_sb", [D, D], F32)
    w2_sb = nc.alloc_sbuf_tensor("w2_sb", [D, D], F32)
    w2bf_sb = nc.alloc_sbuf_tensor("w2bf_sb", [D, D], BF16)
    w3bf_sb = nc.alloc_sbuf_tensor("w3bf_sb", [D, D], BF16)
    v_sb = nc.alloc_sbuf_tensor("v_sb", [128, NB * 128], F32)

    stream_chunks = (_chunk_list(T_lo), _chunk_list(T_hi))

    with tile.TileContext(nc) as tc:
        nc.sync.dma_start(ixpre_sb[:], ixpre_d[:])
        nc.sync.dma_start(ixlo_sb[:], ixlo_d[:])
        if T_hi:
            nc.sync.dma_start(ixhi_sb[:], ixhi_d[:])
        nc.sync.dma_start(xT_sb[:], xT_d[:])
        nc.sync.dma_start(w_sb[:], w_d[:])
        nc.sync.dma_start(wT_sb[:], wT_d[:])

        with (
            tc.tile_pool(name="wps", bufs=2, space="PSUM") as wps,
        ):
            w2_ps = wps.tile([D, D], F32, name="w2_ps")
            nc.tensor.matmul(w2_ps[:], wT_sb[:], w_sb[:], start=True, stop=True)
            nc.vector.tensor_copy(w2_sb[:], w2_ps[:])
            nc.vector.tensor_copy(w2bf_sb[:], w2_ps[:])
            w3_ps = wps.tile([D, D], F32, name="w3_ps")
            nc.tensor.matmul(w3_ps[:], wT_sb[:], w2_sb[:], start=True, stop=True)
            nc.vector.tensor_copy(w3bf_sb[:], w3_ps[:])

        qctr = [0]  # rotate gathers over SWDGE queues: ~2x desc-gen/drain overlap

        def emit_spmm(phase, tab_lo_ap, tab_hi_ap, per_block_tail,
                      pre_gathered=None):
            with (
                tc.tile_pool(name=f"g{phase}", bufs=8) as gpool,
                tc.tile_pool(name=f"m{phase}", bufs=6) as mpool,
                tc.tile_pool(name=f"ps{phase}", bufs=3, space="PSUM") as ppool,
                tc.tile_pool(name=f"tail{phase}", bufs=2, space="PSUM") as tpool,
                tc.tile_pool(name=f"sb{phase}", bufs=3) as spool,
            ):
                gbufs = dict(pre_gathered or {})
                mbufs = {}

                def ensure_chunk(h, ci):
                    k = (h, ci)
                    if k in gbufs:
                        return gbufs[k]
                    t0, ct = stream_chunks[h][ci]
                    n = ct * 128
                    g = gpool.tile([128, CH_TILES, 128], BF16,
                                   tag=f"g{h}", name=f"g{phase}_{h}_{ci}")
                    if phase == 1 and h == 0 and ci == 0:
                        ix = ixpre_sb[:, 0:ct * 8]
                    else:
                        ix = (ixlo_sb, ixhi_sb)[h][:, t0 * 8:(t0 + ct) * 8]
                    tab = (tab_lo_ap, tab_hi_ap)[h]
                    nc.gpsimd.dma_gather(g[:, :ct, :], tab, ix, n, n, D,
                                         queue_num=qctr[0] % 4)
                    qctr[0] += 1
                    gbufs[k] = g
                    return g

                def ensure_mchunk(ci):
                    if ci in mbufs:
                        return mbufs[ci]
                    n = min(MCH, NMM - ci * MCH)
                    mt = mpool.tile([128, MCH * 128], BF16, tag="mk",
                                    name=f"mk{phase}_{ci}")
                    nc.sync.dma_start(
                        mt[:, :n * 128],
                        mask_d[:, ci * MCH * 128:(ci * MCH + n) * 128])
                    mbufs[ci] = mt
                    return mt

                mm_ctr = [0]

                def next_mask():
                    m = mm_ctr[0]
                    mm_ctr[0] += 1
                    mt = ensure_mchunk(m // MCH)
                    off = m % MCH
                    return mt[:, off * 128:(off + 1) * 128]

                for b in range(NB):
                    mms = block_mms[b]
                    ps = ppool.tile([128, 128], F32, tag="ps", name=f"ps{phase}_{b}")
                    for j, (h, t, sl) in enumerate(mms):
                        g = ensure_chunk(h, t // CH_TILES)
                        tic = t % CH_TILES
                        msl = next_mask()
                        first, last = (j == 0), (j == len(mms) - 1)
                        if phase == 1:
                            nc.tensor.matmul(ps[:], g[:, tic, :], msl,
                                             start=first, stop=last)
                        else:
                            nc.tensor.matmul(ps[:], msl, g[:, tic, :],
                                             start=first, stop=last)
                    per_block_tail(b, ps, tpool, spool)

        def tail1(b, ps, tpool, spool):
            rows = min(128, S - 128 * b)
            z1t = spool.tile([128, 128], BF16, tag="z1t", name=f"z1t_{b}")
            nc.scalar.copy(z1t[:], ps[:])                      # ACT [f,d] bf16
            t2_ps = tpool.tile([128, 128], F32, tag="t2ps", name=f"t2ps_{b}")
            nc.tensor.matmul(t2_ps[:], z1t[:], w3bf_sb[:], start=True, stop=True)
            u_ps = tpool.tile([128, 128], F32, tag="ups", name=f"ups_{b}")
            nc.tensor.matmul(u_ps[:], xT_sb[:, b * 128:(b + 1) * 128],
                             w2bf_sb[:], start=True, stop=True)
            t2t = spool.tile([128, 128], BF16, tag="t2t", name=f"t2t_{b}")
            nc.scalar.copy(t2t[:], t2_ps[:])                   # ACT f32->bf16
            nc.vector.tensor_tensor(v_sb[:, b * 128:(b + 1) * 128],
                                    u_ps[:], t2t[:], mybir.AluOpType.add)
            nc.sync.dma_start(cc_in[b * 128:b * 128 + rows, :], t2t[:rows, :])

        def tail2(b, ps, tpool, spool):
            rows = min(128, S - 128 * b)
            y = spool.tile([128, 128], F32, tag="y", name=f"y_{b}")
            nc.vector.tensor_tensor(y[:], ps[:], v_sb[:, b * 128:(b + 1) * 128],
                                    mybir.AluOpType.subtract)
            nc.sync.dma_start(y_d[b * 128:b * 128 + rows, :], y[:rows, :])

        hi_rows = N - SPLIT if N > SPLIT else 0
        tab2_lo = cc_out[0:min(SPLIT, N), :]
        tab2_hi = cc_out[SPLIT:N, :] if hi_rows else None

        emit_spmm(1, tab_d[0:min(SPLIT, N), :],
                  tab_d[SPLIT:N, :] if hi_rows else None, tail1)

        if ncores > 1:
            nc.gpsimd.collective_compute(
                "AllGather", mybir.AluOpType.bypass,
                replica_groups=[list(range(ncores))],
                ins=[cc_in[:]], outs=[cc_out[:]])
        else:
            nc.sync.dma_start(cc_out[:], cc_in[:])

        emit_spmm(2, tab2_lo, tab2_hi, tail2)

    nc.compile()
    return nc


def _make_in_maps(N, ncores, sched, per_core, input_np, W_np):
    S, NB = sched["S"], sched["NB"]
    tab = input_np.astype(ml_dtypes.bfloat16)
    W = W_np.astype(np.float32)
    WT = np.ascontiguousarray(W.T)
    in_maps = []
    for c in range(ncores):
        xT = np.zeros((D, NB * 128), ml_dtypes.bfloat16)
        xT[:, :S] = tab[c * S:(c + 1) * S].T
        m = dict(tab=tab, xT=xT, w=W, wT=WT,
                 ixpre=np.ascontiguousarray(
                     per_core[c]["idx_lo"][:, :CH_TILES * 8]),
                 ixlo=per_core[c]["idx_lo"],
                 mask=per_core[c]["mask"])
        if sched["T_hi"]:
            m["ixhi"] = per_core[c]["idx_hi"]
        in_maps.append(m)
    return in_maps


_cache = {}


def _get_program(N, ncores, sched):
    key = (N, ncores, sched["NMM"], sched["T_lo"], sched["T_hi"])
    if key not in _cache:
        _cache[key] = _build_program(N, ncores, sched)
    return _cache[key]


def run(input, adj_rows, adj_cols, adj_vals, W, ncores=8, trace=False):
    N = input.shape[0]
    sched, per_core = _host_prep(N, ncores, adj_rows, adj_cols, adj_vals)
    nc = _get_program(N, ncores, sched)
    in_maps = _make_in_maps(N, ncores, sched, per_core, np.asarray(input),
                            np.asarray(W))
    res = run_bass_kernel_spmd(nc, in_maps, core_ids=list(range(ncores)),
                               trace=trace)
    y = np.concatenate([res.results[c]["y"] for c in range(ncores)], axis=0)
    return y[:N].astype(np.float32), res


def kernel(input, adj_rows, adj_cols, adj_vals, W):
    y, _ = run(np.asarray(input), np.asarray(adj_rows), np.asarray(adj_cols),
               np.asarray(adj_vals), np.asarray(W), ncores=8)
    return y
